# revision 3
# baseline (speedup 1.0000x reference)
"""Trainium2 Bass kernel for nn_CrossAttention (B=4, Sx=Sy=2048, D=1024, f32).

Sharding: data-parallel over (batch b, query-half h) -> 8 cores; each core
computes attention for 1024 query rows of one batch. The K/V projections are
algebraically folded so NO work is duplicated across cores:

  scores = (x Wq)(y Wk)^T / sqrt(D) = x M y^T,  M = Wq Wk^T / sqrt(D)  (host)
  out    = softmax(scores) (y Wv)   = ((E y) Wv) / rowsum(E),  E = exp(scores)

Per-core device pipeline (s = 1024 queries in two 512-halves, t = 2048 keys):
  m1: A^T[d,s] = M^T-blocks x x^T          (contract e', 2.1 GF)
  m2: S^T[t,s] = y^T-blocks x A^T          (contract d,  4.3 GF)
      E = exp(S - 2)  (softmax shift-invariant; keeps E in fp8 range)
  m3: O^T[d,s] = y-blocks x E^T            (contract t,  4.3 GF)
      r[1,s]   = ones x E^T                (softmax denominator)
  m4: U[s,e]   = O^T-blocks x Wv           (contract d,  2.1 GF)
  host: out = U / (PSCALE * r) + bv

All matmuls run in float8e4 (e4m3) with MatmulPerfMode.DoubleRow (two 128-row
k-tiles per instruction at 0.5 cycles/row). Precision is recovered with hi+lo
splitting: every operand X is stored as X = Xh + Xl (two e4m3 tensors at one
power-of-2 scale), and each GEMM accumulates the three significant products
Xh*Yh + Xh*Yl + Xl*Yh in fp32 PSUM (the dropped Xl*Yl term is ~1e-3 relative).
Weights/activations are pre-split on host; A, E, O are split on device
(Act engine: exp + E-hi copy; DVE: the subtractions and A/O splits).
"""

import numpy as np
import ml_dtypes

import concourse.bacc as bacc
import concourse.bass as bass
import concourse.tile as tile
import concourse.mybir as mybir
from concourse.bass_utils import run_bass_kernel_spmd

F32 = mybir.dt.float32
FP16 = mybir.dt.float16
FP8 = mybir.dt.float8e4
DR = mybir.MatmulPerfMode.DoubleRow
NPF8 = ml_dtypes.float8_e4m3

B, SX, SY, D = 4, 2048, 2048, 1024
NCORES = 8
SXH = SX // 2            # query rows per core
HALF = SXH // 2          # s-half processed as one pipeline stream
DB = D // 128            # 128-row blocks of the d/e' axes
TB = SY // 128           # 128-row blocks of the key axis
XS = 8.0                 # fp8 scale for x
YS = 8.0                 # fp8 scale for y
MS = 8192.0              # fp8 scale for M = Wq Wk^T / sqrt(D)
WS = 256.0               # fp8 scale for Wv
ASH = 2.0 ** -8          # A psum (x65536) -> A8 (x256)
OSH = 2.0 ** -4          # O psum (x8) -> O8 (x0.5)
EBIAS = -2.0             # exp(S + EBIAS): shift-invariant, bounds E by ~33
SPS = XS * MS * ASH * YS     # S psum scale = 2048
PSCALE = YS * OSH * WS       # U psum scale vs r (E is at true scale): 128

_CACHE = {}


def _build():
    nc = bacc.Bacc("TRN2", target_bir_lowering=False, debug=False,
                   num_devices=NCORES, dynamic_dma_scratch_size=2048)

    xth_d = nc.dram_tensor("xth", [DB, 128, SXH], FP8, kind="ExternalInput").ap()
    xtl_d = nc.dram_tensor("xtl", [DB, 128, SXH], FP8, kind="ExternalInput").ap()
    yth_d = nc.dram_tensor("yth", [DB, 128, SY], FP8, kind="ExternalInput").ap()
    ytl_d = nc.dram_tensor("ytl", [DB, 128, SY], FP8, kind="ExternalInput").ap()
    yh_d = nc.dram_tensor("yh", [TB, 128, D], FP8, kind="ExternalInput").ap()
    yl_d = nc.dram_tensor("yl", [TB, 128, D], FP8, kind="ExternalInput").ap()
    # M ships per-d-block partition-major ([dblk][e'-in-block][e'blk*128+d])
    # so each 64KB block DMA moves 1KB contiguous runs at full bus speed
    mh_d = nc.dram_tensor("mh", [DB, 128, D], FP8, kind="ExternalInput").ap()
    ml_d = nc.dram_tensor("ml", [DB, 128, D], FP8, kind="ExternalInput").ap()
    wvh_d = nc.dram_tensor("wvh", [DB, 128, D], FP8, kind="ExternalInput").ap()
    wvl_d = nc.dram_tensor("wvl", [DB, 128, D], FP8, kind="ExternalInput").ap()
    u_d = nc.dram_tensor("u", [SXH, D], FP16, kind="ExternalOutput").ap()
    r_d = nc.dram_tensor("r", [1, SXH], F32, kind="ExternalOutput").ap()

    with tile.TileContext(nc) as tc:
        with (
            tc.tile_pool(name="persist", bufs=1) as persist,
            tc.tile_pool(name="estage", bufs=3) as estage,
            tc.tile_pool(name="ostage", bufs=6) as ostage,
            tc.tile_pool(name="ps", bufs=8, space="PSUM") as psp,
        ):
            ones8 = persist.tile([128, 2, 128], FP8)
            nc.vector.memset(ones8, 1.0)
            ebias_t = persist.tile([128, 1], F32)
            nc.vector.memset(ebias_t, EBIAS)

            xh_t = persist.tile([128, DB, SXH], FP8)
            xl_t = persist.tile([128, DB, SXH], FP8)
            # M per-d-block slabs: [e'-in-block, d-block, e'-block, d-in-blk]
            mh_t = persist.tile([128, DB, DB, 128], FP8)
            ml_t = persist.tile([128, DB, DB, 128], FP8)
            yth_t = persist.tile([128, DB, SY], FP8)
            ytl_t = persist.tile([128, DB, SY], FP8)
            yh_t = persist.tile([128, TB, D], FP8)
            yl_t = persist.tile([128, TB, D], FP8)
            wvh_t = persist.tile([128, DB, D], FP8)
            wvl_t = persist.tile([128, DB, D], FP8)
            ah_t = persist.tile([128, DB, SXH], FP8)   # A^T hi (s-halves packed)
            al_t = persist.tile([128, DB, SXH], FP8)
            eh_t = persist.tile([128, TB, SXH], FP8)   # E^T hi
            el_t = persist.tile([128, TB, SXH], FP8)
            oh_t = persist.tile([128, DB, SXH], FP8)   # O^T hi
            ol_t = persist.tile([128, DB, SXH], FP8)
            rsb = persist.tile([1, SXH], F32)

            def ld(dst, src, sl=None):
                if sl is None:
                    nc.sync.dma_start(out=dst, in_=src.rearrange("b p f -> p b f"))
                else:
                    nc.sync.dma_start(
                        out=dst[:, :, sl],
                        in_=src[:, :, sl].rearrange("b p f -> p b f"))

            # staged prefetch ordered by first use, with few DMAs (each
            # dma_start costs ~650ns of HWDGE issue time): a 64KB M block
            # first so m1's opening matmul starts ~2us in, x halves next,
            # M remainder in two slabs matched to m1's d-block order
            h0, h1 = slice(0, HALF), slice(HALF, SXH)

            def ldm(dst, src, lo, hi):
                nc.sync.dma_start(
                    out=dst[:, lo:hi], in_=src[lo:hi].rearrange("b p f -> p b f"))

            ldm(mh_t, mh_d, 0, 1)
            ld(xh_t, xth_d, h0)
            ld(xl_t, xtl_d, h0)
            ldm(ml_t, ml_d, 0, 1)
            ldm(mh_t, mh_d, 1, 4)
            ldm(ml_t, ml_d, 1, 4)
            ldm(mh_t, mh_d, 4, DB)
            ldm(ml_t, ml_d, 4, DB)
            ld(xh_t, xth_d, h1)
            ld(xl_t, xtl_d, h1)
            ld(yth_t, yth_d)
            ld(ytl_t, ytl_d, slice(0, SY // 2))
            ld(ytl_t, ytl_d, slice(SY // 2, SY))
            ld(yh_t, yh_d)
            ld(yl_t, yl_d)
            ld(wvh_t, wvh_d)
            ld(wvl_t, wvl_d)

            # ---- m1: A^T[d, s] = sum_e' M[e', d-cols]^T x^T[e', s] ----
            for hf in range(2):
                s0 = hf * HALF
                for db in range(DB):
                    ps = psp.tile([128, HALF], F32, tag="ps")
                    steps = [(xh_t, mh_t), (xl_t, mh_t), (xh_t, ml_t)]
                    n = len(steps) * (DB // 2)
                    i = 0
                    for xt, mt in steps:
                        for j in range(DB // 2):
                            nc.tensor.matmul(
                                ps,
                                lhsT=mt[:, db, 2 * j:2 * j + 2, :],
                                rhs=xt[:, 2 * j:2 * j + 2, s0:s0 + HALF],
                                start=(i == 0), stop=(i == n - 1), perf_mode=DR)
                            i += 1
                    nc.scalar.activation(
                        out=ah_t[:, db, s0:s0 + HALF], in_=ps,
                        func=mybir.ActivationFunctionType.Copy, scale=ASH)
                    nc.vector.scalar_tensor_tensor(
                        out=al_t[:, db, s0:s0 + HALF], in0=ps, scalar=ASH,
                        in1=ah_t[:, db, s0:s0 + HALF],
                        op0=mybir.AluOpType.mult,
                        op1=mybir.AluOpType.subtract)

            # ---- m2: S^T[t, s] = sum_d y^T[d, t-cols]^T A^T[d, s]; E=exp ----
            for hf in range(2):
                s0 = hf * HALF
                for tb in range(TB):
                    ps = psp.tile([128, HALF], F32, tag="ps")
                    steps = [(ah_t, yth_t), (al_t, yth_t), (ah_t, ytl_t)]
                    n = len(steps) * (DB // 2)
                    i = 0
                    for at, yt in steps:
                        for j in range(DB // 2):
                            nc.tensor.matmul(
                                ps,
                                lhsT=yt[:, 2 * j:2 * j + 2, tb * 128:(tb + 1) * 128],
                                rhs=at[:, 2 * j:2 * j + 2, s0:s0 + HALF],
                                start=(i == 0), stop=(i == n - 1), perf_mode=DR)
                            i += 1
                    e32 = estage.tile([128, HALF], F32, tag="e32")
                    nc.scalar.activation(
                        out=e32, in_=ps,
                        func=mybir.ActivationFunctionType.Exp,
                        scale=1.0 / SPS, bias=ebias_t)
                    nc.scalar.activation(
                        out=eh_t[:, tb, s0:s0 + HALF], in_=e32,
                        func=mybir.ActivationFunctionType.Copy)
                    nc.vector.tensor_sub(
                        out=el_t[:, tb, s0:s0 + HALF], in0=e32,
                        in1=eh_t[:, tb, s0:s0 + HALF])

            # ---- m3: O^T[d, s] = sum_t y[t, d-cols]^T E^T[t, s]; r = 1^T E ----
            # ---- m4: U[s, e] = sum_d O^T[d, s-cols]^T Wv[d, e] ----
            # phase order m3(A), r(A), m3(B), m4(A), r(B), m4(B): every m4
            # starts >=10us after its O^T splits were issued, so the PE never
            # waits on the Act/DVE split tail, and output DMAs spread out
            def m3_half(hf):
                s0 = hf * HALF
                for db in range(DB):
                    ps = psp.tile([128, HALF], F32, tag="ps")
                    steps = [(eh_t, yh_t), (el_t, yh_t), (eh_t, yl_t)]
                    n = len(steps) * (TB // 2)
                    i = 0
                    for et, yt in steps:
                        for j in range(TB // 2):
                            nc.tensor.matmul(
                                ps,
                                lhsT=yt[:, 2 * j:2 * j + 2, db * 128:(db + 1) * 128],
                                rhs=et[:, 2 * j:2 * j + 2, s0:s0 + HALF],
                                start=(i == 0), stop=(i == n - 1), perf_mode=DR)
                            i += 1
                    nc.scalar.activation(
                        out=oh_t[:, db, s0:s0 + HALF], in_=ps,
                        func=mybir.ActivationFunctionType.Copy, scale=OSH)
                    nc.vector.scalar_tensor_tensor(
                        out=ol_t[:, db, s0:s0 + HALF], in0=ps, scalar=OSH,
                        in1=oh_t[:, db, s0:s0 + HALF],
                        op0=mybir.AluOpType.mult,
                        op1=mybir.AluOpType.subtract)
            def r_half(hf):
                s0 = hf * HALF
                psr = psp.tile([128, HALF], F32, tag="ps")
                i = 0
                for et in (eh_t, el_t):
                    for j in range(TB // 2):
                        nc.tensor.matmul(
                            psr,
                            lhsT=ones8,
                            rhs=et[:, 2 * j:2 * j + 2, s0:s0 + HALF],
                            start=(i == 0), stop=(i == TB - 1), perf_mode=DR)
                        i += 1
                nc.vector.tensor_copy(out=rsb[:, s0:s0 + HALF], in_=psr[0:1, :])

            def m4_half(hf):
                s0 = hf * HALF
                for sb in range(HALF // 128):
                    sc = s0 + sb * 128
                    for ehf in range(2):
                        e0 = ehf * 512
                        ps = psp.tile([128, 512], F32, tag="ps")
                        steps = [(oh_t, wvh_t), (oh_t, wvl_t), (ol_t, wvh_t)]
                        n = len(steps) * (DB // 2)
                        i = 0
                        for ot, wt in steps:
                            for j in range(DB // 2):
                                nc.tensor.matmul(
                                    ps,
                                    lhsT=ot[:, 2 * j:2 * j + 2, sc:sc + 128],
                                    rhs=wt[:, 2 * j:2 * j + 2, e0:e0 + 512],
                                    start=(i == 0), stop=(i == n - 1),
                                    perf_mode=DR)
                                i += 1
                        ub = ostage.tile([128, 512], FP16, tag="u")
                        if ehf == 0:
                            nc.scalar.activation(
                                out=ub, in_=ps,
                                func=mybir.ActivationFunctionType.Copy)
                        else:
                            nc.vector.tensor_copy(out=ub, in_=ps)
                        nc.sync.dma_start(
                            out=u_d[sc:sc + 128, e0:e0 + 512], in_=ub)

            m3_half(0)
            r_half(0)
            m3_half(1)
            m4_half(0)
            r_half(1)
            nc.sync.dma_start(out=r_d, in_=rsb)
            m4_half(1)

    nc.compile()
    return nc


def _get_nc():
    if "nc" not in _CACHE:
        _CACHE["nc"] = _build()
    return _CACHE["nc"]


def _split8(a32, scale):
    """Return (hi, lo) e4m3 arrays with hi + lo ~= a32 * scale."""
    s = (a32 * scale).astype(np.float32)
    hi = s.astype(NPF8)
    lo = (s - hi.astype(np.float32)).astype(NPF8)
    return hi, lo


def make_in_maps(x, y, Wq, bq, Wk, bk, Wv, bv):
    x = np.asarray(x, dtype=np.float32)
    y = np.asarray(y, dtype=np.float32)
    Wq = np.asarray(Wq, dtype=np.float32)
    Wk = np.asarray(Wk, dtype=np.float32)
    Wv = np.asarray(Wv, dtype=np.float32)
    bq = np.asarray(bq, dtype=np.float32)

    s = np.float32(1.0 / np.sqrt(D))
    M = (Wq.astype(np.float64) @ Wk.astype(np.float64).T).astype(np.float32) * s
    assert not np.any(bq), "bq != 0 needs the A-bias path (not built)"

    def m_layout(a):
        # [e'b*128+p, db*128+di] -> [db][p][eb][di] (per-d-block slabs)
        return np.ascontiguousarray(
            a.reshape(DB, 128, DB, 128).transpose(2, 1, 0, 3)
            .reshape(DB, 128, D))  # dram f dim = (eb, di) flattened

    mh, ml = _split8(M, MS)
    mh, ml = m_layout(mh), m_layout(ml)
    wvh, wvl = _split8(Wv, WS)
    wvh = np.ascontiguousarray(wvh.reshape(DB, 128, D))
    wvl = np.ascontiguousarray(wvl.reshape(DB, 128, D))

    per_b = {}
    for b in range(B):
        yt = np.ascontiguousarray(y[b].T)
        yth, ytl = _split8(yt, YS)
        yh, yl = _split8(y[b], YS)
        per_b[b] = {
            "yth": np.ascontiguousarray(yth.reshape(DB, 128, SY)),
            "ytl": np.ascontiguousarray(ytl.reshape(DB, 128, SY)),
            "yh": np.ascontiguousarray(yh.reshape(TB, 128, D)),
            "yl": np.ascontiguousarray(yl.reshape(TB, 128, D)),
        }

    in_maps = []
    for c in range(NCORES):
        b, h = divmod(c, 2)
        xt = np.ascontiguousarray(x[b, h * SXH:(h + 1) * SXH, :].T)
        xth, xtl = _split8(xt, XS)
        in_maps.append({
            "xth": np.ascontiguousarray(xth.reshape(DB, 128, SXH)),
            "xtl": np.ascontiguousarray(xtl.reshape(DB, 128, SXH)),
            "mh": mh, "ml": ml, "wvh": wvh, "wvl": wvl,
            **per_b[b],
        })
    return in_maps


def assemble(results, bv):
    bv = np.asarray(bv, dtype=np.float32)
    out = np.empty((B, SX, D), dtype=np.float32)
    for c in range(NCORES):
        b, h = divmod(c, 2)
        u = results[c]["u"].astype(np.float32)
        r = results[c]["r"].reshape(SXH, 1)
        out[b, h * SXH:(h + 1) * SXH, :] = u / (np.float32(PSCALE) * r)
    out += bv[None, None, :]
    return out


def kernel(x, y, Wq, bq, Wk, bk, Wv, bv):
    nc = _get_nc()
    in_maps = make_in_maps(x, y, Wq, bq, Wk, bk, Wv, bv)
    res = run_bass_kernel_spmd(nc, in_maps, list(range(NCORES)))
    return assemble(res.results, bv)
